# revision 6
# baseline (speedup 1.0000x reference)
"""Trainium2 Bass kernel for nn_MetaModel (moe_routing).

Math: per-ticker MLP states are linear in the M=8 mesa coefficients:
  states[t] = base + bias + meta_W @ mesa_W[:, t]
so with A[t] = [1, mesa_W[:, t]] (9 coeffs):
  w1_eff[t] = sum_m A[t,m] * W1_m,  b1_eff, w2_eff, b2_eff likewise.

Per row n (ticker t=ticker[n]):
  Z[n, 64(m-1)+j] = x_aug[n] @ W1aug_m[j]   m=1..8  (PE, shared weights)
  pre[n, j]   = Z0[n,j] + sum_m A[t,m] * Z[n, ...]  (DVE broadcast scale
                + PE identity-matmul accumulate; the m=0 term is written
                straight into the PRE psum bank by its own matmul)
  h = relu(pre)                                  (ACT)
  q[n, :] = A[t] @ W2aug                          (PE tiny matmul, per tile)
  out[n]  = h_aug[n] . q[n]                       (DVE mult + reduce,
                pipelined two tiles behind so no same-engine RAW stall)

No indirect gathers: the per-row coefficients A[t_n] (9 floats) are laid
out host-side into dense tensors (AS row-major for the scale, ART
transposed for the w2 matmul), like the baseline's tickT marshaling.
Weight tables are host-summed (base+bias), so there is no device phase 0.
Data parallel over N=32768 rows across 8 cores (4096 rows each).
"""
import sys

sys.path.insert(0, "/opt/trn_rl_repo")
import numpy as np

from concourse.bass_utils import run_bass_kernel_spmd
from concourse import bass, mybir

F32 = mybir.dt.float32
BF16 = mybir.dt.bfloat16
AF = mybir.ActivationFunctionType
ALU = mybir.AluOpType

D, H, T, M, N, S = 32, 64, 1024, 8, 32768, 2177
NCORES = 8
R = N // NCORES          # rows per core = 4096
NT = R // 128            # tiles per core = 32
KA = D + 1               # 33 (ones-augmented contraction)
ZW = 8 * H               # 512 (scaled blocks m=1..8)
W2W = H + 1              # 65

last_results = None      # test.py reads trace info from here

_cached = None


def _build_program():
    nc = bass.Bass()

    xT = nc.dram_tensor("xT", [KA, R], BF16, kind="ExternalInput")
    wc = nc.dram_tensor("wc", [KA, ZW + H], BF16, kind="ExternalInput")
    w2t = nc.dram_tensor("w2t", [9, W2W], BF16, kind="ExternalInput")
    art = nc.dram_tensor("art", [9, R], BF16, kind="ExternalInput")
    ass = nc.dram_tensor("ass", [128, NT * 8], BF16, kind="ExternalInput")
    ident = nc.dram_tensor("ident", [128, 128], BF16, kind="ExternalInput")
    y = nc.dram_tensor("y", [128, NT], F32, kind="ExternalOutput")

    from contextlib import ExitStack
    with ExitStack() as ctx:
        e = ctx.enter_context
        # sbuf
        XT = e(nc.sbuf_tensor([KA, R], BF16))
        WCs = e(nc.sbuf_tensor([KA, ZW + H], BF16))
        W2s = e(nc.sbuf_tensor([9, W2W], BF16))
        ARTs = e(nc.sbuf_tensor([9, R], BF16))
        ASs = e(nc.sbuf_tensor([128, NT * 8], BF16))
        IDN = e(nc.sbuf_tensor([128, 128], BF16))
        AM = e(nc.sbuf_tensor([128, 2 * ZW], BF16))
        HB = e(nc.sbuf_tensor([128, 3 * W2W], BF16))
        TMP = e(nc.sbuf_tensor([128, 2 * W2W], F32))
        OUT = e(nc.sbuf_tensor([128, NT], F32))
        # psum: one bank per tensor; readers/writers of concurrently-live
        # tiles always hit different banks.
        ZA0 = e(nc.psum_tensor([128, ZW], F32))
        ZA1 = e(nc.psum_tensor([128, ZW], F32))
        P0 = e(nc.psum_tensor([128, H], F32))
        P1 = e(nc.psum_tensor([128, H], F32))
        Q0 = e(nc.psum_tensor([128, W2W], F32))
        Q1 = e(nc.psum_tensor([128, W2W], F32))
        Q2 = e(nc.psum_tensor([128, W2W], F32))
        ZAP = [ZA0, ZA1]
        PP = [P0, P1]
        QP = [Q0, Q1, Q2]
        # semaphores
        s_w = e(nc.semaphore("s_w"))        # const DMAs on sync queue (wc)
        s_k = e(nc.semaphore("s_k"))        # const DMAs on gpsimd queue
        s_x = [e(nc.semaphore(f"s_x{c}")) for c in range(4)]
        s_z = e(nc.semaphore("s_z"))        # ZA matmul done (per tile)
        s_q = e(nc.semaphore("s_q"))        # w2g matmul done (per tile)
        s_pre = e(nc.semaphore("s_pre"))    # combine done (per tile)
        s_sc = e(nc.semaphore("s_sc"))      # scale done (per tile)
        s_h = e(nc.semaphore("s_h"))        # relu done (per tile)
        s_o = e(nc.semaphore("s_o"))        # w2-dot done (per tile)
        s_y = e(nc.semaphore("s_y"))
        block = e(nc.Block())

        @block.sync
        def _(sync):
            sync.dma_start(out=WCs[:], in_=wc[:]).then_inc(s_w, 16)
            CH = R // 4
            for c in range(4):
                sync.dma_start(
                    out=XT[:, c * CH:(c + 1) * CH], in_=xT[:, c * CH:(c + 1) * CH]
                ).then_inc(s_x[c], 16)
            sync.wait_ge(s_o, NT)
            sync.dma_start(out=y[:], in_=OUT[:]).then_inc(s_y, 16)
            sync.wait_ge(s_y, 16)

        @block.gpsimd
        def _(gp):
            gp.dma_start(out=ASs[:], in_=ass[:]).then_inc(s_k, 16)
            gp.dma_start(out=ARTs[:], in_=art[:]).then_inc(s_k, 16)
            gp.dma_start(out=W2s[:], in_=w2t[:]).then_inc(s_k, 16)
            gp.dma_start(out=IDN[:], in_=ident[:]).then_inc(s_k, 16)

        @block.tensor
        def _(te):
            for i in range(NT + 1):
                b = i % 2
                pb = (i - 1) % 2
                if i < NT:
                    if i % 8 == 0:
                        te.wait_ge(s_x[i // 8], 16)
                    if i == 0:
                        te.wait_ge(s_w, 16)
                        te.wait_ge(s_k, 64)
                    lt = XT[:, i * 128:(i + 1) * 128]
                    if i >= 2:
                        te.wait_ge(s_sc, i - 1)   # ZA[b] free (scale(i-2))
                    nc.tensor.matmul(ZAP[b][:], lhsT=lt, rhs=WCs[:, 0:ZW],
                                     start=True, stop=True).then_inc(s_z, 1)
                    if i >= 2:
                        te.wait_ge(s_h, i - 1)    # PRE[b] free (relu(i-2))
                    nc.tensor.matmul(PP[b][:], lhsT=lt, rhs=WCs[:, ZW:ZW + H],
                                     start=True, stop=False,
                                     skip_group_check=True)
                    if i >= 3:
                        te.wait_ge(s_o, i - 2)    # Q[i%3] free (dot(i-3))
                    nc.tensor.matmul(QP[i % 3][:],
                                     lhsT=ARTs[:, i * 128:(i + 1) * 128],
                                     rhs=W2s[:], start=True,
                                     stop=True).then_inc(s_q, 1)
                if i >= 1:
                    te.wait_ge(s_sc, i)           # scale(i-1) done
                    for m in range(8):
                        op = nc.tensor.matmul(
                            PP[pb][:], lhsT=IDN[:],
                            rhs=AM[:, pb * ZW + m * H: pb * ZW + (m + 1) * H],
                            start=False, stop=(m == 7),
                            skip_group_check=True)
                    op.then_inc(s_pre, 1)

        @block.vector
        def _(ve):
            # per iteration: mult(j=i-2) | scale(i) | reduce(j).  The big
            # scale op sits between the same-engine RAW pair mult->reduce
            # (TMP); the tail iterations (no scale) use a drain instead.
            nc.vector.memset(HB[:, H:H + 1], 1.0)
            nc.vector.memset(HB[:, W2W + H:W2W + H + 1], 1.0)
            nc.vector.memset(HB[:, 2 * W2W + H:2 * W2W + H + 1], 1.0)
            ve.wait_ge(s_k, 16)  # ASs loaded (first gpsimd DMA)
            for i in range(NT + 2):
                b = i % 2
                j = i - 2
                if j >= 0:
                    ve.wait_ge(s_h, j + 1)        # relu(j) done
                    ve.wait_ge(s_q, j + 1)        # w2g(j) done
                    nc.vector.tensor_tensor(
                        out=TMP[:, (j % 2) * W2W:(j % 2 + 1) * W2W],
                        in0=HB[:, (j % 3) * W2W:(j % 3 + 1) * W2W],
                        in1=QP[j % 3][:], op=ALU.mult)
                if i < NT:
                    ve.wait_ge(s_z, i + 1)        # ZA(i) done
                    if i >= 2:
                        ve.wait_ge(s_pre, i - 1)  # AM[b] free (combine(i-2))
                    nc.vector.tensor_tensor(
                        out=AM[:, b * ZW:(b + 1) * ZW].rearrange(
                            "p (m j) -> p m j", j=H),
                        in0=ZAP[b][:].rearrange("p (m j) -> p m j", j=H),
                        in1=ASs[:, i * 8:(i + 1) * 8].unsqueeze(2)
                        .broadcast_to((128, 8, H)),
                        op=ALU.mult,
                    ).then_inc(s_sc, 1)
                elif j >= 0:
                    ve.drain()
                if j >= 0:
                    nc.vector.tensor_reduce(
                        out=OUT[:, j:j + 1],
                        in_=TMP[:, (j % 2) * W2W:(j % 2 + 1) * W2W],
                        axis=mybir.AxisListType.X, op=ALU.add,
                    ).then_inc(s_o, 1)

        @block.scalar
        def _(act):
            for i in range(NT):
                act.wait_ge(s_pre, i + 1)         # combine(i) done
                if i >= 3:
                    act.wait_ge(s_o, i - 2)       # HB[i%3] free (dot(i-3))
                nc.scalar.activation(
                    out=HB[:, (i % 3) * W2W: (i % 3) * W2W + H], in_=PP[i % 2][:],
                    func=AF.Relu,
                ).then_inc(s_h, 1)

    return nc


def _host_prep(x, ticker, mesa_w, meta_w, meta_b, base):
    f32 = np.float32
    import ml_dtypes
    bf16 = ml_dtypes.bfloat16

    # 9-basis state stack: m=0 -> base + bias, m=1..8 -> meta_W columns
    st = np.zeros((9, S), f32)
    st[0] = base + meta_b
    st[1:] = meta_w.T

    # wc: [33, 512+64] — cols 0..511 are blocks m=1..8, cols 512.. m=0
    wcf = np.zeros((KA, ZW + H), f32)
    for m in range(9):
        c0 = (m - 1) * H if m >= 1 else ZW
        blk = st[m, :H * D].reshape(H, D)
        wcf[0:D, c0:c0 + H] = blk.T
        wcf[D, c0:c0 + H] = st[m, H * D:H * D + H]
    wc = wcf.astype(bf16)

    # w2t: [9, 65] — rows m, cols [w2_m | b2_m]
    w2f = np.zeros((9, W2W), f32)
    w2f[:, 0:H] = st[:, H * D + H:H * D + H + H]
    w2f[:, H] = st[:, S - 1]
    w2t = w2f.astype(bf16)

    ident = np.eye(128, dtype=bf16)

    # per-row mesa coefficients, [8, N] f32
    Arows = mesa_w[:, ticker]                     # [8, N]

    shared = dict(wc=wc, w2t=w2t, ident=ident)
    in_maps = []
    for c in range(NCORES):
        rows = slice(c * R, (c + 1) * R)
        xt = np.empty((KA, R), bf16)
        xt[0:D] = x[rows].T
        xt[D] = 1.0
        artc = np.zeros((9, R), bf16)
        artc[0] = 1.0
        artc[1:9] = Arows[:, rows]
        # ass[p, i*8+k] = A_{k+1}(row i*128+p)
        assc = np.ascontiguousarray(
            Arows[:, rows].reshape(8, NT, 128).transpose(2, 1, 0)
            .reshape(128, NT * 8)).astype(bf16)
        in_maps.append(dict(xT=np.ascontiguousarray(xt),
                            art=np.ascontiguousarray(artc),
                            ass=assc, **shared))
    return in_maps


def kernel(x, ticker, mesa_layer_weight, meta_layer_weight, meta_layer_bias,
           base_state):
    global _cached, last_results
    if _cached is None:
        _cached = _build_program()
    nc = _cached
    in_maps = _host_prep(
        np.asarray(x, np.float32), np.asarray(ticker),
        np.asarray(mesa_layer_weight, np.float32),
        np.asarray(meta_layer_weight, np.float32),
        np.asarray(meta_layer_bias, np.float32),
        np.asarray(base_state, np.float32))
    res = run_bass_kernel_spmd(nc, in_maps, core_ids=list(range(NCORES)))
    last_results = res
    out = np.empty((N, 1), np.float32)
    for c in range(NCORES):
        yc = res.results[c]["y"]              # [128, NT]
        out[c * R:(c + 1) * R, 0] = yc.T.reshape(R)
    return out


# revision 9
# speedup vs baseline: 1.1768x; 1.1768x over previous
"""Trainium2 Bass kernel for nn_MetaModel (moe_routing).

Math: per-ticker MLP states are linear in the M=8 mesa coefficients:
  states[t] = base + bias + meta_W @ mesa_W[:, t]
so with A[t] = [1, mesa_W[:, t]] (9 coeffs):
  w1_eff[t] = sum_m A[t,m] * W1_m,  b1_eff, w2_eff, b2_eff likewise.

Per row n (ticker t=ticker[n]):
  Z[n, 64(m-1)+j] = x_aug[n] @ W1aug_m[j]   m=1..8  (PE, shared weights)
  pre[n, j]   = Z0[n,j] + sum_m A[t,m] * Z[n, ...]  (DVE broadcast scale
                + PE identity-matmul accumulate; the m=0 term is written
                straight into the PRE psum bank by its own matmul)
  h = relu(pre)                                  (ACT)
  q[n, :] = A[t] @ W2aug                          (PE tiny matmul, per tile)
  out[n]  = h_aug[n] . q[n]                       (DVE mult, GpSimd reduce)

No indirect gathers: the per-row coefficients A[t_n] (9 floats) are laid
out host-side into dense tensors (AS row-major for the scale, ART
transposed for the w2 matmul), like the baseline's tickT marshaling.
Weight tables are host-summed (base+bias), so there is no device phase 0.
PSUM: ZA x2 | PRE x3 | Q x3 = 8 banks; the 3-deep PRE/Q rings keep the
PE free of relu/mult round-trip stalls.
Data parallel over N=32768 rows across 8 cores (4096 rows each).
"""
import sys

sys.path.insert(0, "/opt/trn_rl_repo")
import numpy as np

from concourse.bass_utils import run_bass_kernel_spmd
from concourse import bass, mybir

F32 = mybir.dt.float32
BF16 = mybir.dt.bfloat16
AF = mybir.ActivationFunctionType
ALU = mybir.AluOpType

D, H, T, M, N, S = 32, 64, 1024, 8, 32768, 2177
NCORES = 8
R = N // NCORES          # rows per core = 4096
NT = R // 128            # tiles per core = 32
KA = D + 1               # 33 (ones-augmented contraction)
ZW = 8 * H               # 512 (scaled blocks m=1..8)
W2W = H + 1              # 65

last_results = None      # test.py reads trace info from here

_cached = None


def _build_program():
    nc = bass.Bass()

    xT = nc.dram_tensor("xT", [KA, R], BF16, kind="ExternalInput")
    wc = nc.dram_tensor("wc", [KA, ZW + H], BF16, kind="ExternalInput")
    w2t = nc.dram_tensor("w2t", [9, W2W], BF16, kind="ExternalInput")
    art = nc.dram_tensor("art", [9, R], BF16, kind="ExternalInput")
    ass = nc.dram_tensor("ass", [128, NT * 8], BF16, kind="ExternalInput")
    ident = nc.dram_tensor("ident", [128, 128], BF16, kind="ExternalInput")
    y = nc.dram_tensor("y", [128, NT], F32, kind="ExternalOutput")

    from contextlib import ExitStack
    with ExitStack() as ctx:
        e = ctx.enter_context
        # sbuf
        XT = e(nc.sbuf_tensor([KA, R], BF16))
        WCs = e(nc.sbuf_tensor([KA, ZW + H], BF16))
        W2s = e(nc.sbuf_tensor([9, W2W], BF16))
        ARTs = e(nc.sbuf_tensor([9, R], BF16))
        ASs = e(nc.sbuf_tensor([128, NT * 8], BF16))
        IDN = e(nc.sbuf_tensor([128, 128], BF16))
        AM = e(nc.sbuf_tensor([128, 2 * ZW], BF16))
        HB = e(nc.sbuf_tensor([128, 3 * W2W], F32))
        QS = e(nc.sbuf_tensor([128, 2 * W2W], F32))
        TMP = e(nc.sbuf_tensor([128, 2 * W2W], F32))
        OUT = e(nc.sbuf_tensor([128, NT], F32))
        # psum: whole banks per tensor; readers/writers of concurrently
        # live tiles always hit different banks.
        ZA0 = e(nc.psum_tensor([128, ZW], F32))
        ZA1 = e(nc.psum_tensor([128, ZW], F32))
        P0 = e(nc.psum_tensor([128, H], F32))
        P1 = e(nc.psum_tensor([128, H], F32))
        P2 = e(nc.psum_tensor([128, H], F32))
        Q0 = e(nc.psum_tensor([128, W2W], F32))
        Q1 = e(nc.psum_tensor([128, W2W], F32))
        Q2 = e(nc.psum_tensor([128, W2W], F32))
        ZAP = [ZA0, ZA1]
        PP = [P0, P1, P2]
        QP = [Q0, Q1, Q2]
        # semaphores
        s_w = e(nc.semaphore("s_w"))        # wc DMA (scalar queue)
        s_k = e(nc.semaphore("s_k"))        # const DMAs on gpsimd queue
        s_x = [e(nc.semaphore(f"s_x{c}")) for c in range(4)]
        s_z = e(nc.semaphore("s_z"))        # ZA matmul done (per tile)
        s_q = e(nc.semaphore("s_q"))        # w2g matmul done (per tile)
        s_pre = e(nc.semaphore("s_pre"))    # combine done (per tile)
        s_sc = e(nc.semaphore("s_sc"))      # scale done (per tile)
        s_h = e(nc.semaphore("s_h"))        # relu done (per tile)
        s_t = e(nc.semaphore("s_t"))        # w2 dot reduce done (per tile)
        s_m = e(nc.semaphore("s_m"))        # w2 mult done (per tile)
        s_qc = e(nc.semaphore("s_qc"))      # Q psum->sbuf copy done
        s_y = e(nc.semaphore("s_y"))
        block = e(nc.Block())

        @block.sync
        def _(sync):
            CH = R // 4
            for c in range(4):
                sync.dma_start(
                    out=XT[:, c * CH:(c + 1) * CH], in_=xT[:, c * CH:(c + 1) * CH]
                ).then_inc(s_x[c], 16)
            sync.wait_ge(s_t, NT)
            sync.dma_start(out=y[:], in_=OUT[:]).then_inc(s_y, 16)
            sync.wait_ge(s_y, 16)

        @block.gpsimd
        def _(gp):
            gp.dma_start(out=ASs[:], in_=ass[:]).then_inc(s_k, 16)
            gp.dma_start(out=ARTs[:], in_=art[:]).then_inc(s_k, 16)
            gp.dma_start(out=W2s[:], in_=w2t[:]).then_inc(s_k, 16)
            gp.dma_start(out=IDN[:], in_=ident[:]).then_inc(s_k, 16)

        @block.tensor
        def _(te):
            for i in range(NT + 1):
                if i < NT:
                    if i % 8 == 0:
                        te.wait_ge(s_x[i // 8], 16)
                    if i == 0:
                        te.wait_ge(s_w, 16)
                    lt = XT[:, i * 128:(i + 1) * 128]
                    if i >= 2:
                        te.wait_ge(s_sc, i - 1)   # ZA[i%2] free (scale(i-2))
                    nc.tensor.matmul(ZAP[i % 2][:], lhsT=lt, rhs=WCs[:, 0:ZW],
                                     start=True, stop=True).then_inc(s_z, 1)
                    if i >= 3:
                        te.wait_ge(s_h, i - 2)    # PRE[i%3] free (relu(i-3))
                    nc.tensor.matmul(PP[i % 3][:], lhsT=lt, rhs=WCs[:, ZW:ZW + H],
                                     start=True, stop=False,
                                     skip_group_check=True)
                    if i == 0:
                        te.wait_ge(s_k, 48)       # ARTs + W2s loaded
                    if i >= 3:
                        te.wait_ge(s_qc, i - 2)   # Q[i%3] free (qcopy(i-3))
                    nc.tensor.matmul(QP[i % 3][:],
                                     lhsT=ARTs[:, i * 128:(i + 1) * 128],
                                     rhs=W2s[:], start=True,
                                     stop=True).then_inc(s_q, 1)
                if i >= 1:
                    if i == 1:
                        te.wait_ge(s_k, 64)       # IDN loaded
                    pb = (i - 1) % 3
                    te.wait_ge(s_sc, i)           # scale(i-1) done
                    for m in range(8):
                        op = nc.tensor.matmul(
                            PP[pb][:], lhsT=IDN[:],
                            rhs=AM[:, ((i - 1) % 2) * ZW + m * H:
                                   ((i - 1) % 2) * ZW + (m + 1) * H],
                            start=False, stop=(m == 7),
                            skip_group_check=True)
                    op.then_inc(s_pre, 1)

        @block.vector
        def _(ve):
            nc.vector.memset(HB[:, H:H + 1], 1.0)
            nc.vector.memset(HB[:, W2W + H:W2W + H + 1], 1.0)
            nc.vector.memset(HB[:, 2 * W2W + H:2 * W2W + H + 1], 1.0)
            ve.wait_ge(s_k, 16)  # ASs loaded (first gpsimd DMA)
            for i in range(NT + 2):
                j = i - 2
                if j >= 0:
                    ve.wait_ge(s_h, j + 1)        # relu(j) done
                    ve.wait_ge(s_qc, j + 1)       # qcopy(j) done
                    nc.vector.tensor_tensor(
                        out=TMP[:, (j % 2) * W2W:(j % 2 + 1) * W2W],
                        in0=HB[:, (j % 3) * W2W:(j % 3 + 1) * W2W],
                        in1=QS[:, (j % 2) * W2W:(j % 2 + 1) * W2W],
                        op=ALU.mult,
                    ).then_inc(s_m, 1)
                if i < NT:
                    ve.wait_ge(s_z, i + 1)        # ZA(i) done
                    if i >= 2:
                        ve.wait_ge(s_pre, i - 1)  # AM[i%2] free (combine(i-2))
                    nc.vector.tensor_tensor(
                        out=AM[:, (i % 2) * ZW:(i % 2 + 1) * ZW].rearrange(
                            "p (m j) -> p m j", j=H),
                        in0=ZAP[i % 2][:].rearrange("p (m j) -> p m j", j=H),
                        in1=ASs[:, i * 8:(i + 1) * 8].unsqueeze(2)
                        .broadcast_to((128, 8, H)),
                        op=ALU.mult,
                    ).then_inc(s_sc, 1)
                elif j >= 0:
                    ve.drain()
                if j >= 0:
                    nc.vector.tensor_reduce(
                        out=OUT[:, j:j + 1],
                        in_=TMP[:, (j % 2) * W2W:(j % 2 + 1) * W2W],
                        axis=mybir.AxisListType.X, op=ALU.add,
                    ).then_inc(s_t, 1)

        @block.scalar
        def _(act):
            act.dma_start(out=WCs[:], in_=wc[:]).then_inc(s_w, 16)
            for i in range(NT):
                act.wait_ge(s_q, i + 1)           # w2g(i) done
                if i >= 2:
                    act.wait_ge(s_m, i - 1)       # QS[i%2] free (mult(i-2))
                nc.scalar.activation(
                    out=QS[:, (i % 2) * W2W:(i % 2 + 1) * W2W],
                    in_=QP[i % 3][:], func=AF.Copy,
                ).then_inc(s_qc, 1)
                act.wait_ge(s_pre, i + 1)         # combine(i) done
                if i >= 3:
                    act.wait_ge(s_m, i - 2)       # HB[i%3] free (mult(i-3))
                nc.scalar.activation(
                    out=HB[:, (i % 3) * W2W: (i % 3) * W2W + H],
                    in_=PP[i % 3][:],
                    func=AF.Relu,
                ).then_inc(s_h, 1)

    return nc


def _host_prep(x, ticker, mesa_w, meta_w, meta_b, base):
    f32 = np.float32
    import ml_dtypes
    bf16 = ml_dtypes.bfloat16

    # 9-basis state stack: m=0 -> base + bias, m=1..8 -> meta_W columns
    st = np.zeros((9, S), f32)
    st[0] = base + meta_b
    st[1:] = meta_w.T

    # wc: [33, 512+64] — cols 0..511 are blocks m=1..8, cols 512.. m=0
    wcf = np.zeros((KA, ZW + H), f32)
    for m in range(9):
        c0 = (m - 1) * H if m >= 1 else ZW
        blk = st[m, :H * D].reshape(H, D)
        wcf[0:D, c0:c0 + H] = blk.T
        wcf[D, c0:c0 + H] = st[m, H * D:H * D + H]
    wc = wcf.astype(bf16)

    # w2t: [9, 65] — rows m, cols [w2_m | b2_m]
    w2f = np.zeros((9, W2W), f32)
    w2f[:, 0:H] = st[:, H * D + H:H * D + H + H]
    w2f[:, H] = st[:, S - 1]
    w2t = w2f.astype(bf16)

    ident = np.eye(128, dtype=bf16)

    # per-row mesa coefficients, [8, N] f32
    Arows = mesa_w[:, ticker]                     # [8, N]

    shared = dict(wc=wc, w2t=w2t, ident=ident)
    in_maps = []
    for c in range(NCORES):
        rows = slice(c * R, (c + 1) * R)
        xt = np.empty((KA, R), bf16)
        xt[0:D] = x[rows].T
        xt[D] = 1.0
        artc = np.zeros((9, R), bf16)
        artc[0] = 1.0
        artc[1:9] = Arows[:, rows]
        # ass[p, i*8+k] = A_{k+1}(row i*128+p)
        assc = np.ascontiguousarray(
            Arows[:, rows].reshape(8, NT, 128).transpose(2, 1, 0)
            .reshape(128, NT * 8)).astype(bf16)
        in_maps.append(dict(xT=np.ascontiguousarray(xt),
                            art=np.ascontiguousarray(artc),
                            ass=assc, **shared))
    return in_maps


def kernel(x, ticker, mesa_layer_weight, meta_layer_weight, meta_layer_bias,
           base_state):
    global _cached, last_results
    if _cached is None:
        _cached = _build_program()
    nc = _cached
    in_maps = _host_prep(
        np.asarray(x, np.float32), np.asarray(ticker),
        np.asarray(mesa_layer_weight, np.float32),
        np.asarray(meta_layer_weight, np.float32),
        np.asarray(meta_layer_bias, np.float32),
        np.asarray(base_state, np.float32))
    res = run_bass_kernel_spmd(nc, in_maps, core_ids=list(range(NCORES)))
    last_results = res
    out = np.empty((N, 1), np.float32)
    for c in range(NCORES):
        yc = res.results[c]["y"]              # [128, NT]
        out[c * R:(c + 1) * R, 0] = yc.T.reshape(R)
    return out
